# revision 95
# baseline (speedup 1.0000x reference)
"""Trainium2 Bass kernel for grouped-query causal attention (B=2, T=2048, C=1024,
16 q heads / 4 kv heads, RoPE, fused qkv + output projection).

Sharding: 8 cores = (batch b, kv-head h). Each core:
  - projects x -> qT (4 heads), kT, vT with pre-sliced/pre-scaled bf16 weights
    (transposed layout: channels on partitions, T on free dim)
  - applies RoPE (pair-swap via permutation matmul on PE + DVE mul/add)
  - causal attention for its 4 query heads: S^T blocks with width restricted
    to the causal region on diagonal blocks, a constant upper-triangle bias
    (-50) accumulated into S via matmul (so exp(S) is ~0 where masked; no
    post-exp mask op), softmax denominators via a ones column appended to V
  - partial output projection y^T = Wf_local^T @ oT  (bf16, transposed)
Host sums the 4 per-h partials per batch, adds b_final, transposes back.
"""

import sys

sys.path.insert(0, "/opt/trn_rl_repo")

import ml_dtypes
import numpy as np

import concourse.bacc as bacc
import concourse.mybir as mybir
from concourse import tile
from concourse.bass_utils import run_bass_kernel_spmd

B, T, C = 2, 2048, 1024
G, HKV, HS = 4, 4, 64
OUT_DIM = C + 2 * (C // G)
SCALE = 1.0 / np.sqrt(HS)
MAX_PERIOD = 10000.0
NEG = -50.0  # causal mask bias; logits are O(8) so exp(s + NEG) ~ 0

F32 = mybir.dt.float32
F32R = mybir.dt.float32r
BF16 = mybir.dt.bfloat16
AF = mybir.ActivationFunctionType


TCH = T // 512  # 4 chunks of 512 along T
NT = T // 128  # 16 tiles of 128 along T


def build_nc():
    nc = bacc.Bacc(None, target_bir_lowering=False)

    # partition-major, chunk-contiguous layouts (8KB DMA lines -> full BW)
    xT_d = nc.dram_tensor("xT", [128, TCH, 8, 512], BF16, kind="ExternalInput")
    w_d = nc.dram_tensor("w_qkv", [128, 8, 384], BF16, kind="ExternalInput")
    bl_d = nc.dram_tensor("b_loc", [128, 3], F32, kind="ExternalInput")
    cos_d = nc.dram_tensor("cosT", [128, T], BF16, kind="ExternalInput")
    sin_d = nc.dram_tensor("sinT", [128, T], BF16, kind="ExternalInput")
    perm_d = nc.dram_tensor("perm", [128, 128], BF16, kind="ExternalInput")
    ones_d = nc.dram_tensor("onesd", [128, 64], F32R, kind="ExternalInput")
    eye_d = nc.dram_tensor("eye64", [128, 64], BF16, kind="ExternalInput")
    mtri_d = nc.dram_tensor("mtri", [128, 2, 128], BF16, kind="ExternalInput")
    i128_d = nc.dram_tensor("i128", [128, 128], BF16, kind="ExternalInput")
    wf_d = nc.dram_tensor("wf", [128, 2, 1024], BF16, kind="ExternalInput")
    yT_d = nc.dram_tensor("yT", [C, T], BF16, kind="ExternalOutput")

    with tile.TileContext(nc) as tc:
        with (
            tc.tile_pool(name="persist", bufs=1) as pp,
            tc.tile_pool(name="xstream", bufs=3) as spx,
            tc.tile_pool(name="pstream", bufs=19) as spp,
            tc.tile_pool(name="rstream", bufs=3) as spr,
            tc.tile_pool(name="ostream", bufs=3) as spo,
            tc.tile_pool(name="ps_acc", bufs=2, space="PSUM") as psacc,
            tc.tile_pool(name="ps_s", bufs=2, space="PSUM") as pss,
            tc.tile_pool(name="ps_tmp", bufs=1, space="PSUM") as ps,
        ):
            # ---- persistent tiles ----
            # w in two tiles: first proj matmuls start after half the bytes
            w_lo = pp.tile([128, 4, 384], BF16, tag="wlo", name="wlo")
            w_hi = pp.tile([128, 4, 384], BF16, tag="whi", name="whi")
            bl_sb = pp.tile([128, 3], F32, tag="bl", name="bl")
            cos_sb = pp.tile([128, T], BF16, tag="cos", name="cos")
            sin_sb = pp.tile([128, T], BF16, tag="sin", name="sin")
            # chunk-0 rope tables in their own small tiles: rope(0) must not
            # wait for the full 1MB cos+sin transfers
            cos0_sb = pp.tile([128, 512], BF16, tag="cos0", name="cos0")
            sin0_sb = pp.tile([128, 512], BF16, tag="sin0", name="sin0")
            perm_sb = pp.tile([128, 128], BF16, tag="perm", name="perm")
            ones_sb = pp.tile([128, 64], F32R, tag="ones", name="ones")
            eye_sb = pp.tile([128, 64], BF16, tag="eye", name="eye")
            mtri_sb = pp.tile([128, 2, 128], BF16, tag="mtri", name="mtri")
            i128_sb = pp.tile([128, 128], BF16, tag="i128", name="i128")
            wf_sb = pp.tile([128, 2, 1024], BF16, tag="wf", name="wf")
            qkvT = [pp.tile([128, T], BF16, tag=f"qkvT{m}", name=f"qkvT{m}") for m in range(3)]
            qcat = [pp.tile([64, 2, T], BF16, tag=f"qcat{m}", name=f"qcat{m}") for m in range(2)]
            v_sb = pp.tile([128, NT, 65], BF16, tag="vaug", name="vaug")
            oT_ab = [pp.tile([128, T], BF16, tag=f"oT{i}", name=f"oT{i}") for i in range(2)]

            nc.sync.dma_start(bl_sb[:], bl_d[:])
            nc.gpsimd.memset(v_sb[:, :, 64:65], 1.0)
            # preload the Exp table set while the prologue DMAs run
            warm_sb = pp.tile([128, 1], BF16, tag="warm", name="warm")
            nc.scalar.activation(warm_sb[0:1, 0:1], bl_sb[0:1, 0:1], AF.Exp)

            # -- emission helpers ------------------------------------------
            def load_x(tc_i):
                xt = spx.tile([128, 8, 512], BF16, tag="xt", name="xt")
                nc.sync.dma_start(xt[:, 0:4, :], xT_d[:, tc_i, 0:4, :])

                def rest():
                    # second half deferred so small DMAs can interleave
                    nc.sync.dma_start(xt[:, 4:8, :], xT_d[:, tc_i, 4:8, :])

                return xt, rest

            def proj_tasks(tc_i, getx, first=False):
                """Dense background tasks for chunk tc_i's projection+RoPE+vT."""
                tsl = slice(tc_i * 512, (tc_i + 1) * 512)
                csrc = cos0_sb if tc_i == 0 else cos_sb
                ssrc = sin0_sb if tc_i == 0 else sin_sb
                csl = slice(0, 512) if tc_i == 0 else tsl

                def mk_group(mt):
                    # split into 2-matmul subtasks so PE bursts stay short
                    prh = []

                    def mk_sub(k0):
                        def run():
                            if k0 == 0:
                                if first and mt > 0:
                                    # prologue: borrow idle attention PSUM so
                                    # all 3 groups can accumulate at once
                                    prh.append(
                                        pss.tile([128, 512], F32, tag="s", name="prs")
                                    )
                                else:
                                    prh.append(
                                        ps.tile([128, 512], F32, tag="prj", name="prj")
                                    )
                            pr = prh[0]
                            for k in (k0, k0 + 1):
                                wt = w_lo if k < 4 else w_hi
                                nc.tensor.matmul(
                                    pr[:],
                                    wt[:, k % 4, mt * 128 : (mt + 1) * 128],
                                    getx(k),
                                    start=(k == 0),
                                    stop=(k == 7),
                                )
                            if k0 == 6:
                                nc.vector.tensor_scalar_add(
                                    qkvT[mt][:, tsl], pr[:], bl_sb[:, mt : mt + 1]
                                )
                        return run

                    return [mk_sub(k0) for k0 in (0, 2, 4, 6)]

                def mk_rope_q(mt):
                    def run():
                        tmp = ps.tile([128, 512], F32, tag="tmp", name="tmp")
                        nc.tensor.matmul(
                            tmp[:], perm_sb[:], qkvT[mt][:, tsl], start=True, stop=True
                        )
                        nc.vector.tensor_mul(
                            qkvT[mt][:, tsl], qkvT[mt][:, tsl], csrc[:, csl]
                        )
                        tmpb = spp.tile([128, 1024], BF16, tag="p", name="p")
                        nc.vector.tensor_mul(tmpb[:, 0:512], tmp[:], ssrc[:, csl])
                        nc.vector.tensor_add(
                            qkvT[mt][:, tsl], qkvT[mt][:, tsl], tmpb[:, 0:512]
                        )
                        nc.sync.dma_start(qcat[mt][:, 0, tsl], qkvT[mt][0:64, tsl])
                        nc.sync.dma_start(qcat[mt][:, 1, tsl], qkvT[mt][64:128, tsl])
                    return run

                def mk_vt(i):
                    def run():
                        tt = tc_i * 4 + i
                        vt = ps.tile([128, 512], BF16, tag="tmp", name="tmp")
                        nc.tensor.transpose(
                            vt[:, 0:64],
                            qkvT[2][64:128, tt * 128 : (tt + 1) * 128],
                            eye_sb[64:128, :],
                        )
                        nc.vector.tensor_copy(v_sb[:, tt, 0:64], vt[:, 0:64])
                    return run

                def rope_k():
                    tmp = ps.tile([128, 512], F32, tag="tmp", name="tmp")
                    nc.tensor.matmul(
                        tmp[0:64, :], perm_sb[:, 0:64], qkvT[2][:, tsl],
                        start=True, stop=True,
                    )
                    nc.vector.tensor_mul(
                        qkvT[2][0:64, tsl], qkvT[2][0:64, tsl], csrc[0:64, csl]
                    )
                    tmpb = spp.tile([128, 1024], BF16, tag="p", name="p")
                    nc.vector.tensor_mul(
                        tmpb[0:64, 0:512], tmp[0:64, :], ssrc[0:64, csl]
                    )
                    nc.vector.tensor_add(
                        qkvT[2][0:64, tsl], qkvT[2][0:64, tsl], tmpb[0:64, 0:512]
                    )

                g0, g1, g2 = mk_group(0), mk_group(1), mk_group(2)
                if first:
                    # x arrives in halves: run every group's k0-3 first
                    mmtasks = [
                        g0[0], g0[1], g1[0], g1[1], g2[0], g2[1],
                        g0[2], g0[3], g1[2], g1[3], g2[2], g2[3],
                    ]
                else:
                    mmtasks = g0 + g1 + g2
                return (
                    mmtasks
                    + [mk_rope_q(0), mk_rope_q(1)]
                    + [mk_vt(0), mk_vt(1), mk_vt(2), mk_vt(3)]
                    + [rope_k]
                )

            def final_tasks(t0, width):
                # y partials in bf16 PSUM (one bank even at width=1024)
                tsl = slice(t0, t0 + width)

                def mk(nt):
                    def run():
                        # alternate PSUM slots + copy engine so consecutive
                        # tiles double-buffer instead of serializing
                        y_ps = ps.tile(
                            [128, 512], F32,
                            tag=("tmp" if nt % 2 == 0 else "prj"), name="yps",
                        )
                        for cc in range(2):
                            nc.tensor.matmul(
                                y_ps[:, 0:width],
                                wf_sb[:, cc, nt * 128 : (nt + 1) * 128],
                                oT_ab[cc][:, tsl],
                                start=(cc == 0),
                                stop=(cc == 1),
                            )
                        y_sb = spo.tile([128, 512], BF16, tag="yout", name="yout")
                        if nt % 2 == 0:
                            nc.vector.tensor_copy(y_sb[:, 0:width], y_ps[:, 0:width])
                        else:
                            nc.scalar.activation(
                                y_sb[:, 0:width], y_ps[:, 0:width], AF.Copy
                            )
                        nc.sync.dma_start(
                            yT_d[nt * 128 : (nt + 1) * 128, tsl], y_sb[:, 0:width]
                        )
                    return run

                return [mk(nt) for nt in range(8)]

            def final_epilogue():
                # last 512 columns, split by contraction half: cc0 reads only
                # oT_ab[0] so it can run before the final norms land
                tiles = {}

                def cc0(nt):
                    y_ps = ps.tile(
                        [128, 512], F32,
                        tag=("tmp" if nt % 2 == 0 else "prj"), name="yps",
                    )
                    tiles[nt] = y_ps
                    nc.tensor.matmul(
                        y_ps[:],
                        wf_sb[:, 0, nt * 128 : (nt + 1) * 128],
                        oT_ab[0][:, 1536:2048],
                        start=True,
                        stop=False,
                    )

                def cc1(nt):
                    y_ps = tiles[nt]
                    nc.tensor.matmul(
                        y_ps[:],
                        wf_sb[:, 1, nt * 128 : (nt + 1) * 128],
                        oT_ab[1][:, 1536:2048],
                        start=False,
                        stop=True,
                    )
                    y_sb = spo.tile([128, 512], BF16, tag="yout", name="yout")
                    if nt % 2 == 0:
                        nc.vector.tensor_copy(y_sb[:], y_ps[:])
                    else:
                        nc.scalar.activation(y_sb[:], y_ps[:], AF.Copy)
                    nc.sync.dma_start(
                        yT_d[nt * 128 : (nt + 1) * 128, 1536:2048], y_sb[:]
                    )

                return cc0, cc1

            # -- prologue: contiguous full-BW loads; chunk-0 x AND w in two
            # separate tiles each so projection starts after half the bytes
            nc.sync.dma_start(w_lo[:], w_d[:, 0:4, :])
            xta = spx.tile([128, 4, 512], BF16, tag="xta", name="xta")
            nc.sync.dma_start(xta[:], xT_d[:, 0, 0:4, :])
            nc.sync.dma_start(w_hi[:], w_d[:, 4:8, :])
            xtb = spx.tile([128, 4, 512], BF16, tag="xtb", name="xtb")
            nc.sync.dma_start(xtb[:], xT_d[:, 0, 4:8, :])
            getx0 = lambda k: (xta if k < 4 else xtb)[:, k % 4, :]
            nc.sync.dma_start(cos0_sb[:], cos_d[:, 0:512])
            nc.sync.dma_start(sin0_sb[:], sin_d[:, 0:512])
            nc.sync.dma_start(cos_sb[:, 512:2048], cos_d[:, 512:2048])
            nc.sync.dma_start(sin_sb[:, 512:2048], sin_d[:, 512:2048])
            nc.sync.dma_start(perm_sb[:], perm_d[:])
            nc.scalar.dma_start(mtri_sb[:], mtri_d[:])
            nc.scalar.dma_start(i128_sb[:], i128_d[:])
            nc.scalar.dma_start(eye_sb[:], eye_d[:])
            nc.scalar.dma_start(wf_sb[:], wf_d[:])
            nc.scalar.dma_start(ones_sb[:], ones_d[:])
            for t in proj_tasks(0, getx0, first=True):
                t()

            # -- main loop -------------------------------------------------
            pending_norm = []  # deferred norm-finish closures

            def mk_norm(g, tci, o_ac, fast=False):
                tsl = slice(tci * 512, (tci + 1) * 512)
                odd = g % 2 == 1

                def run():
                    if fast:
                        # end-of-kernel: PE is idle, broadcast via matmul
                        # (shorter latency than the gpsimd chain)
                        s1 = spr.tile([128, 512], F32R, tag="inv", name="inv")
                        nc.vector.tensor_copy(s1[64:65, :], o_ac[64:65, :])
                        bc = pss.tile([128, 2, 512], F32, tag="s", name="s")
                        nc.tensor.matmul(
                            bc[0:64, 0, :], ones_sb[64:65, 0:64], s1[64:65, :],
                            start=True, stop=True,
                        )
                        bc_sb = spr.tile([64, 512], F32, tag="bcr", name="bcr")
                        nc.vector.reciprocal_approx_fast(
                            out=bc_sb[:], in_=bc[0:64, 0, :]
                        )
                    else:
                        # sums live on partition 64 of PSUM; partition_broadcast
                        # reads PHYSICAL partition 0 and 1-partition reciprocal
                        # is broken on HW, so: copy -> DMA hop 64->0 ->
                        # broadcast raw sums -> reciprocal on 64 partitions.
                        s1 = spr.tile([128, 512], F32, tag="inv", name="inv")
                        nc.vector.tensor_copy(s1[64:65, :], o_ac[64:65, :])
                        nc.sync.dma_start(s1[0:1, :], s1[64:65, :])
                        bcsum = spr.tile([64, 512], F32, tag="bcs", name="bcs")
                        nc.gpsimd.partition_broadcast(bcsum[:], s1[0:1, :])
                        bc_sb = spr.tile([64, 512], F32, tag="bcr", name="bcr")
                        nc.vector.reciprocal_approx_fast(out=bc_sb[:], in_=bcsum[:])
                    if odd:
                        stg = spr.tile([64, 512], BF16, tag="stg", name="stg")
                        nc.vector.tensor_mul(stg[:], o_ac[0:64, :], bc_sb[:])
                        nc.sync.dma_start(oT_ab[g // 2][64:128, tsl], stg[:])
                    else:
                        nc.vector.tensor_mul(
                            oT_ab[g // 2][0:64, tsl], o_ac[0:64, :], bc_sb[:]
                        )
                return run

            for tci in range(TCH):
                tsl = slice(tci * 512, (tci + 1) * 512)
                nblk = 4 * tci + 4

                bg = []
                if tci + 1 < TCH:
                    xts, xrest = load_x(tci + 1)
                    pt = proj_tasks(tci + 1, lambda k, xts=xts: xts[:, k, :])
                    bg += pt[:2] + [xrest] + pt[2:]
                if tci == 2:
                    bg += final_tasks(0, 512)
                elif tci == 3:
                    bg += final_tasks(512, 512) + final_tasks(1024, 512)
                bg_done = 0
                bg_total = len(bg)
                slots = 2 * nblk
                slot = 0

                for pair in range(2):
                    qc = qcat[pair]
                    o_acs = [
                        psacc.tile([128, 512], F32, tag="oacc", name="oacc")
                        for _ in range(2)
                    ]
                    DEPTH = 4
                    pq = []  # (j, q0, p_tile) waiting for PV

                    def emit_pv(jj, qq0, p_tile, o_acs=o_acs, nblk=nblk, heads=(0, 1)):
                        for h01 in heads:
                            nc.tensor.matmul(
                                o_acs[h01][0:65, qq0:512],
                                v_sb[:, jj, 0:65],
                                p_tile[:, h01, qq0:512],
                                start=(jj == 0),
                                stop=(jj == nblk - 1),
                            )

                    for j in range(nblk):
                        # norm chains first: they free the o_acc banks the
                        # next pair's first PV will need
                        if pending_norm:
                            pending_norm.pop(0)()
                        # ready PV next so it isn't queued behind a blocked S
                        if len(pq) >= DEPTH:
                            jj, qq0, pv = pq.pop(0)
                            emit_pv(jj, qq0, pv)
                        d = j - 4 * tci
                        q0 = max(0, 128 * d)
                        s_ps = pss.tile([128, 2, 512], F32, tag="s", name="s")
                        if d >= 0:
                            # constant upper-triangle bias, accumulated first
                            for h01 in range(2):
                                nc.tensor.matmul(
                                    s_ps[:, h01, q0 : q0 + 128],
                                    i128_sb[:],
                                    mtri_sb[:, h01, :],
                                    start=True,
                                    stop=False,
                                )
                        for h01 in range(2):
                            nc.tensor.matmul(
                                s_ps[:, h01, q0:512],
                                qkvT[2][0:64, j * 128 : (j + 1) * 128],
                                qc[:, h01, tci * 512 + q0 : (tci + 1) * 512],
                                start=(d < 0),
                                stop=True,
                            )
                        p_sb = spp.tile([128, 2, 512], BF16, tag="p", name="p")
                        nc.scalar.activation(
                            p_sb[:, :, q0:512], s_ps[:, :, q0:512], AF.Exp
                        )
                        pq.append((j, q0, p_sb))
                        slot += 1
                        due = bg_total * min(slot, slots) // slots
                        while bg_done < due:
                            bg[bg_done]()
                            bg_done += 1

                    if tci == TCH - 1 and pair == 1:
                        # last pair: finish h1 first and norm eagerly via the
                        # matmul-broadcast path so the final tasks start sooner
                        while bg_done < bg_total:
                            bg[bg_done]()
                            bg_done += 1
                        for jj, qq0, pv in pq:
                            emit_pv(jj, qq0, pv, heads=(1,))
                        mk_norm(pair * 2 + 1, tci, o_acs[1], fast=True)()
                        # cc0 finals only need oT_ab[0]: start them now
                        cc0_f, cc1_f = final_epilogue()
                        cc0_f(0)
                        cc0_f(1)
                        for jj, qq0, pv in pq:
                            emit_pv(jj, qq0, pv, heads=(0,))
                        mk_norm(pair * 2, tci, o_acs[0], fast=True)()
                    else:
                        for jj, qq0, pv in pq:
                            emit_pv(jj, qq0, pv)
                        for h01 in range(2):
                            g = pair * 2 + h01
                            pending_norm.append(mk_norm(g, tci, o_acs[h01]))

                while bg_done < bg_total:
                    bg[bg_done]()
                    bg_done += 1

            for fn in pending_norm:
                fn()
            # interleave cc1 (frees the PSUM slot) with the next cc0
            cc1_f(0)
            for nt in range(2, 8):
                cc0_f(nt)
                cc1_f(nt - 1)
            cc1_f(7)

    nc.compile()
    return nc


def host_shard(inputs):
    """Build the 8 per-core input maps from full inputs."""
    x = np.ascontiguousarray(np.asarray(inputs["input"], dtype=np.float32))
    W = np.asarray(inputs["W_attn"], dtype=np.float32)
    bb = np.asarray(inputs["b_attn"], dtype=np.float32)
    Wf = np.asarray(inputs["W_final"], dtype=np.float32)

    half = HS // 2
    inv_freq = MAX_PERIOD ** (-np.arange(half, dtype=np.float32) / half)
    ang = np.arange(T, dtype=np.float32)[:, None] * inv_freq  # (T, 32)
    sin_t = np.sin(ang).astype(np.float32)
    cos_t = np.cos(ang).astype(np.float32)
    cosT = np.repeat(cos_t.T, 2, axis=0)  # (64, T): row d -> cos(t*f[d//2])
    sgn = np.where(np.arange(HS) % 2 == 0, -1.0, 1.0).astype(np.float32)
    sinT = np.repeat(sin_t.T, 2, axis=0) * sgn[:, None]
    cos128 = np.ascontiguousarray(np.concatenate([cosT, cosT], axis=0))
    sin128 = np.ascontiguousarray(np.concatenate([sinT, sinT], axis=0))

    perm = np.zeros((128, 128), np.float32)
    idx = np.arange(128)
    perm[idx ^ 1, idx] = 1.0
    eye64 = np.zeros((128, 64), np.float32)
    eye64[64:128, :] = np.eye(64, dtype=np.float32)
    # mtri[k, h, q] = NEG where (local) q < k, else 0 (same for both heads)
    kk = np.arange(128)
    tri = np.where(kk[None, :] < kk[:, None], NEG, 0.0).astype(np.float32)
    mtri = np.ascontiguousarray(np.stack([tri, tri], axis=1))  # (128, 2, 128)
    i128 = np.eye(128, dtype=np.float32)

    in_maps = []
    for cid in range(8):
        b, h = cid // 4, cid % 4
        qcols = np.concatenate(
            [np.arange(g * 256 + h * 64, g * 256 + h * 64 + 64) for g in range(G)]
        )
        kcols = np.arange(1024 + h * 64, 1024 + h * 64 + 64)
        vcols = np.arange(1280 + h * 64, 1280 + h * 64 + 64)
        cols = np.concatenate([qcols, kcols, vcols])
        w_loc = W[:, cols].copy()
        b_loc = bb[cols].copy()
        w_loc[:, :256] *= SCALE
        b_loc[:256] *= SCALE
        b_loc_m = np.ascontiguousarray(b_loc.reshape(3, 128).T)  # (128, 3)

        rows = np.concatenate(
            [np.arange(g * 256 + h * 64, g * 256 + h * 64 + 64) for g in range(G)]
        )
        # partition-major contiguous layouts for full-bandwidth DMA
        wf_loc = np.ascontiguousarray(
            Wf[rows, :].reshape(2, 128, 1024).transpose(1, 0, 2)
        )  # (128, 2, 1024)
        xT_m = np.ascontiguousarray(
            x[b].T.reshape(8, 128, TCH, 512).transpose(1, 2, 0, 3)
        )  # (128, TCH, 8, 512)
        w_m = np.ascontiguousarray(
            w_loc.reshape(8, 128, 384).transpose(1, 0, 2)
        )  # (128, 8, 384)

        in_maps.append(
            {
                "xT": xT_m.astype(ml_dtypes.bfloat16),
                "w_qkv": w_m.astype(ml_dtypes.bfloat16),
                "b_loc": b_loc_m,
                "cosT": cos128.astype(ml_dtypes.bfloat16),
                "sinT": sin128.astype(ml_dtypes.bfloat16),
                "perm": perm.astype(ml_dtypes.bfloat16),
                "onesd": np.ones((128, 64), np.float32),
                "eye64": eye64.astype(ml_dtypes.bfloat16),
                "mtri": mtri.astype(ml_dtypes.bfloat16),
                "i128": i128.astype(ml_dtypes.bfloat16),
                "wf": wf_loc.astype(ml_dtypes.bfloat16),
            }
        )
    return in_maps


def host_unshard(results, b_final):
    """Sum the 4 per-h partial yT per batch, add b_final, transpose back."""
    out = np.empty((B, T, C), np.float32)
    for b in range(B):
        acc = results[b * 4]["yT"].astype(np.float32)
        for h in range(1, 4):
            acc = acc + results[b * 4 + h]["yT"]
        out[b] = acc.T + b_final[None, :]
    return out


_NC_CACHE = None


def _get_nc():
    global _NC_CACHE
    if _NC_CACHE is None:
        _NC_CACHE = build_nc()
    return _NC_CACHE


def kernel(**inputs):
    nc = _get_nc()
    in_maps = host_shard(inputs)
    res = run_bass_kernel_spmd(nc, in_maps, core_ids=list(range(8)))
    b_final = np.asarray(inputs["b_final"], dtype=np.float32)
    return host_unshard(res.results, b_final)
